# revision 7
# baseline (speedup 1.0000x reference)
"""VQ EuclideanCodebook forward on 8 Trainium2 NeuronCores.

Strategy (data-parallel over batch):
  - x [8, 8192, 128] is sharded by batch: core b gets x[b] (8192 tokens).
  - The [1024, 128] codebook is replicated on every core.
  - Per core, per 128-token tile:
      PE:   d = 2*x @ embed.T - ||e||^2  via fp32 matmuls (exact) plus an
            exact -||e||^2 bias as a K=3 bf16 rank-reduction matmul
            (bf16 hi/mid/lo split of esq, 1 cycle/row).
      ACT:  PSUM -> SBUF extraction of the distance tile.
      DVE:  InstMax (top-8) + InstMaxIndex -> argmax index per token.
      POOL: indirect DMA gather embed[idx] -> quantized tile, store out.
  - argmax(2*x@e.T - ||e||^2) == argmax of the reference distance
    -(||x||^2 - 2 x@e.T + ||e||^2) exactly (per-token constant shift).

kernel(x, embed) -> (quantized [8,8192,128] f32, indices [8,8192] int32)
"""
import sys
sys.path.insert(0, "/opt/trn_rl_repo")
from contextlib import ExitStack
import numpy as np
import ml_dtypes

import concourse.bass as bass
import concourse.mybir as mybir
from concourse.bass_utils import run_bass_kernel_spmd

F32 = mybir.dt.float32
BF16 = mybir.dt.bfloat16
U32 = mybir.dt.uint32
I32 = mybir.dt.int32

B, TOK, C = 8, 8192, 128
K = 1024
NT = TOK // 128          # 64 tiles per core

NX = 4                   # x tile slots
NPS = 4                  # psum slots (4 x 2 banks = all 8 banks)
ND = 4                   # distance SBUF slots
NQ = 4                   # quantized tile slots
NIX = 4                  # index tile slots


def build(nc: bass.Bass):
    xT = nc.declare_dram_parameter("xT", [C, TOK], F32, isOutput=False)
    embT2 = nc.declare_dram_parameter("embT2", [C, K], F32, isOutput=False)
    esq3 = nc.declare_dram_parameter("esq3", [3, K], BF16, isOutput=False)
    ones3 = nc.declare_dram_parameter("ones3", [3, 128], BF16, isOutput=False)
    embed = nc.declare_dram_parameter("embed", [K, C], F32, isOutput=False)

    q_out = nc.declare_dram_parameter("q", [TOK, C], F32, isOutput=True)
    ind_out = nc.declare_dram_parameter("ind", [128, NT, 8], I32, isOutput=True)

    ctx = ExitStack()
    with ctx:
        embT2_s = ctx.enter_context(nc.sbuf_tensor([C, K], F32))
        esq_s = ctx.enter_context(nc.sbuf_tensor([3, K], BF16))
        ones_s = ctx.enter_context(nc.sbuf_tensor([3, 128], BF16))
        x_sb = [ctx.enter_context(nc.sbuf_tensor(f"x_sb{b}", [C, 128], F32))
                for b in range(NX)]
        d_sb = [ctx.enter_context(nc.sbuf_tensor(f"d_sb{b}", [128, K], F32))
                for b in range(ND)]
        mx_sb = [ctx.enter_context(nc.sbuf_tensor(f"mx_sb{b}", [128, 8], F32))
                 for b in range(4)]
        idx_stage = ctx.enter_context(nc.sbuf_tensor([128, NT * 8], U32))
        act_warm = ctx.enter_context(nc.sbuf_tensor([1, 1], F32))
        pe_warm = ctx.enter_context(nc.sbuf_tensor([1, 512], BF16))
        warm_sem = ctx.enter_context(nc.semaphore("warm"))
        q_sb = [ctx.enter_context(nc.sbuf_tensor(f"q_sb{b}", [128, C], F32))
                for b in range(NQ)]
        d_ps = [ctx.enter_context(nc.psum_tensor(f"d_ps{b}", [128, K], F32))
                for b in range(NPS)]

        cemb_sem = ctx.enter_context(nc.semaphore("cemb"))
        cbias_sem = ctx.enter_context(nc.semaphore("cbias"))
        xin_sems = [ctx.enter_context(nc.semaphore(f"xin{b}")) for b in range(NX)]
        pe_sem = ctx.enter_context(nc.semaphore("pe"))
        ext_sem = ctx.enter_context(nc.semaphore("ext"))
        dve_sem = ctx.enter_context(nc.semaphore("dve"))
        mx_sem = ctx.enter_context(nc.semaphore("mx"))
        gat_sems = [ctx.enter_context(nc.semaphore(f"gat{b}")) for b in range(NQ)]
        qout_sems = [ctx.enter_context(nc.semaphore(f"qout{b}")) for b in range(NQ)]
        out_sem = ctx.enter_context(nc.semaphore("out"))

        block = ctx.enter_context(nc.Block())

        @block.sync
        def _(sync):
            sync.dma_start(x_sb[0][:], xT[:, 0:128]).then_inc(xin_sems[0], 16)
            sync.dma_start(embT2_s[:], embT2[:]).then_inc(cemb_sem, 16)
            sync.dma_start(esq_s[:], esq3[:]).then_inc(cbias_sem, 16)
            sync.dma_start(ones_s[:], ones3[:]).then_inc(cbias_sem, 16)
            for i in range(1, NT):
                if i >= NX:
                    # slot reuse: PE must be done with tile i-NX
                    sync.wait_ge(pe_sem, i - NX + 1)
                sync.dma_start(x_sb[i % NX][:], xT[:, i * 128:(i + 1) * 128]) \
                    .then_inc(xin_sems[i % NX], 16)
            sync.wait_ge(dve_sem, NT)
            sync.dma_start(ind_out[:], idx_stage[:].bitcast(I32)) \
                .then_inc(out_sem, 16)

        @block.tensor
        def _(tensor):
            tensor.wait_ge(warm_sem, 1)
            for _ in range(16):
                nc.tensor.matmul(d_ps[0][0:1, 0:512], lhsT=pe_warm[:, 0:1],
                                 rhs=pe_warm[:], start=True, stop=True)
            tensor.wait_ge(cemb_sem, 16)
            tensor.wait_ge(cbias_sem, 32)
            for i in range(NT):
                tensor.wait_ge(xin_sems[i % NX], 16 * (i // NX + 1))
                if i >= NPS:
                    tensor.wait_ge(ext_sem, i - NPS + 1)
                ps = d_ps[i % NPS]
                xs = x_sb[i % NX]
                for j in range(2):
                    sl = slice(j * 512, (j + 1) * 512)
                    nc.tensor.matmul(ps[:, sl], lhsT=xs[:], rhs=embT2_s[:, sl],
                                     start=True, stop=False)
                for j in range(2):
                    sl = slice(j * 512, (j + 1) * 512)
                    mm = nc.tensor.matmul(ps[:, sl], lhsT=ones_s[:],
                                          rhs=esq_s[:, sl], start=False, stop=True)
                    if j == 1:
                        mm.then_inc(pe_sem, 1)

        @block.scalar
        def _(scalar):
            scalar.wait_ge(cbias_sem, 32)
            nc.scalar.copy(act_warm[:], ones_s[0:1, 0:1])
            for i in range(NT):
                scalar.wait_ge(pe_sem, i + 1)
                if i >= ND:
                    # d slot reuse: DVE must be done with tile i-ND
                    scalar.wait_ge(dve_sem, i - ND + 1)
                nc.scalar.copy(d_sb[i % ND][:], d_ps[i % NPS][:]) \
                    .then_inc(ext_sem, 1)

        @block.vector
        def _(vector):
            nc.vector.memset(pe_warm[:], 0.0)
            nc.vector.drain()
            vector.sem_inc(warm_sem, 1)
            # software-pipelined: max(i) runs between max_index(i-1)'s producer
            # max(i-1) and its read of mx_sb -- the intervening ~1us op drains
            # the DVE pipeline, so no explicit drain is needed.
            for i in range(NT):
                vector.wait_ge(ext_sem, i + 1)
                if i >= 4:
                    # mx slot WAR: max_index(i-4) must have read mx_sb[i%4]
                    vector.wait_ge(dve_sem, i - 3)
                nc.vector.max(out=mx_sb[i % 4][:], in_=d_sb[i % ND][:]) \
                    .then_inc(mx_sem, 1)
                if i >= 1:
                    j = i - 1
                    vector.wait_ge(mx_sem, j + 1)
                    nc.vector.max_index(out=idx_stage[:, 8 * j:8 * (j + 1)],
                                        in_max=mx_sb[j % 4][:],
                                        in_values=d_sb[j % ND][:]) \
                        .then_inc(dve_sem, 1)
            j = NT - 1
            vector.wait_ge(mx_sem, j + 1)
            nc.vector.max_index(out=idx_stage[:, 8 * j:8 * (j + 1)],
                                in_max=mx_sb[j % 4][:],
                                in_values=d_sb[j % ND][:]) \
                .then_inc(dve_sem, 1)

        @block.gpsimd
        def _(gpsimd):
            for i in range(NT):
                gpsimd.wait_ge(dve_sem, i + 1)
                if i >= NQ:
                    gpsimd.wait_ge(qout_sems[i % NQ], 16 * (i // NQ))
                nc.gpsimd.indirect_dma_start(
                    out=q_sb[i % NQ][:], out_offset=None, in_=embed[:],
                    in_offset=bass.IndirectOffsetOnAxis(
                        ap=idx_stage[:, 8 * i:8 * i + 1], axis=0)
                ).then_inc(gat_sems[i % NQ], 16)
                gpsimd.wait_ge(gat_sems[i % NQ], 16 * (i // NQ + 1))
                nc.gpsimd.dma_start(q_out[i * 128:(i + 1) * 128, :],
                                    q_sb[i % NQ][:]).then_inc(qout_sems[i % NQ], 16)
    return nc


_CACHE = {}


def _get_nc():
    if "nc" not in _CACHE:
        nc = bass.Bass()
        build(nc)
        _CACHE["nc"] = nc
    return _CACHE["nc"]


def kernel(x, embed):
    x = np.asarray(x, dtype=np.float32)
    embed = np.asarray(embed, dtype=np.float32)
    assert x.shape == (B, TOK, C) and embed.shape == (K, C)

    embT2 = np.ascontiguousarray((2.0 * embed).T).astype(np.float32)
    esq = (embed.astype(np.float64) ** 2).sum(1)
    # exact -esq as sum of three bf16 terms
    t = -esq
    e1 = t.astype(ml_dtypes.bfloat16)
    t1 = t - e1.astype(np.float64)
    e2 = t1.astype(ml_dtypes.bfloat16)
    t2 = t1 - e2.astype(np.float64)
    e3 = t2.astype(ml_dtypes.bfloat16)
    esq3 = np.stack([e1, e2, e3]).astype(ml_dtypes.bfloat16)
    ones3 = np.ones((3, 128), ml_dtypes.bfloat16)

    nc = _get_nc()
    in_maps = []
    for b in range(B):
        xTb = np.ascontiguousarray(x[b].T)
        in_maps.append(dict(xT=xTb, embT2=embT2, esq3=esq3, ones3=ones3,
                            embed=embed))
    res = run_bass_kernel_spmd(nc, in_maps, core_ids=list(range(B)))

    quant = np.empty((B, TOK, C), np.float32)
    ind = np.empty((B, TOK), np.int32)
    for b in range(B):
        out = res.results[b]
        quant[b] = out["q"]
        ind[b] = out["ind"][:, :, 0].T.reshape(TOK)
    return quant, ind


# revision 17
# speedup vs baseline: 1.0049x; 1.0049x over previous
"""VQ EuclideanCodebook forward on 8 Trainium2 NeuronCores.

Strategy (data-parallel over batch):
  - x [8, 8192, 128] is sharded by batch: core b gets x[b] (8192 tokens).
  - The [1024, 128] codebook is replicated on every core.
  - Per core, per 128-token tile:
      PE:   d = 2*x @ embed.T - ||e||^2  via fp32 matmuls (exact) plus an
            exact -||e||^2 bias as a K=3 bf16 rank-reduction matmul
            (bf16 hi/mid/lo split of esq, 1 cycle/row).
      ACT:  PSUM -> SBUF extraction of the distance tile.
      DVE:  InstMax (top-8) + InstMaxIndex -> argmax index per token.
      POOL: indirect DMA gather embed[idx] -> quantized tile, store out.
  - argmax(2*x@e.T - ||e||^2) == argmax of the reference distance
    -(||x||^2 - 2 x@e.T + ||e||^2) exactly (per-token constant shift).

kernel(x, embed) -> (quantized [8,8192,128] f32, indices [8,8192] int32)
"""
import sys
sys.path.insert(0, "/opt/trn_rl_repo")
from contextlib import ExitStack
import numpy as np
import ml_dtypes

import concourse.bass as bass
import concourse.mybir as mybir
from concourse.bass_utils import run_bass_kernel_spmd

F32 = mybir.dt.float32
BF16 = mybir.dt.bfloat16
U32 = mybir.dt.uint32
I32 = mybir.dt.int32

B, TOK, C = 8, 8192, 128
K = 1024
NT = TOK // 128          # 64 tiles per core

NX = 8                   # x tile slots
NPS = 4                  # psum slots (4 x 2 banks = all 8 banks)
ND = 8                   # distance SBUF slots
NQ = 8                   # quantized tile slots
SLAG = 8                 # q-store lag behind x-loads on the SP stream


def build(nc: bass.Bass):
    xT = nc.declare_dram_parameter("xT", [C, TOK], F32, isOutput=False)
    embT2 = nc.declare_dram_parameter("embT2", [C, K], F32, isOutput=False)
    esq3 = nc.declare_dram_parameter("esq3", [3, K], BF16, isOutput=False)
    ones3 = nc.declare_dram_parameter("ones3", [3, 128], BF16, isOutput=False)
    embed = nc.declare_dram_parameter("embed", [K, C], F32, isOutput=False)

    q_out = nc.declare_dram_parameter("q", [TOK, C], F32, isOutput=True)
    ind_out = nc.declare_dram_parameter("ind", [128, NT, 8], I32, isOutput=True)

    ctx = ExitStack()
    with ctx:
        embT2_s = ctx.enter_context(nc.sbuf_tensor([C, K], F32))
        esq_s = ctx.enter_context(nc.sbuf_tensor([3, K], BF16))
        ones_s = ctx.enter_context(nc.sbuf_tensor([3, 128], BF16))
        x_sb = [ctx.enter_context(nc.sbuf_tensor(f"x_sb{b}", [C, 128], F32))
                for b in range(NX)]
        d_sb = [ctx.enter_context(nc.sbuf_tensor(f"d_sb{b}", [128, K], F32))
                for b in range(ND)]
        mx_sb = [ctx.enter_context(nc.sbuf_tensor(f"mx_sb{b}", [128, 8], F32))
                 for b in range(4)]
        idx_stage = ctx.enter_context(nc.sbuf_tensor([128, NT * 8], U32))
        act_warm = ctx.enter_context(nc.sbuf_tensor([1, 1], F32))
        pe_warm = ctx.enter_context(nc.sbuf_tensor([1, 512], BF16))
        warm_sem = ctx.enter_context(nc.semaphore("warm"))
        q_sb = [ctx.enter_context(nc.sbuf_tensor(f"q_sb{b}", [128, C], F32))
                for b in range(NQ)]
        d_ps = [ctx.enter_context(nc.psum_tensor(f"d_ps{b}", [128, K], F32))
                for b in range(NPS)]

        cemb_sems = [ctx.enter_context(nc.semaphore(f"cemb{b}")) for b in range(2)]
        cbias_sem = ctx.enter_context(nc.semaphore("cbias"))
        xin_sems = [ctx.enter_context(nc.semaphore(f"xin{b}")) for b in range(NX)]
        pe_sem = ctx.enter_context(nc.semaphore("pe"))
        ext_sem = ctx.enter_context(nc.semaphore("ext"))
        dve_sem = ctx.enter_context(nc.semaphore("dve"))
        mx_sem = ctx.enter_context(nc.semaphore("mx"))
        gat_sems = [ctx.enter_context(nc.semaphore(f"gat{b}")) for b in range(NQ)]
        qout_sems = [ctx.enter_context(nc.semaphore(f"qout{b}")) for b in range(NQ)]
        out_sem = ctx.enter_context(nc.semaphore("out"))

        block = ctx.enter_context(nc.Block())

        @block.sync
        def _(sync):
            sync.dma_start(x_sb[0][:], xT[:, 0:128]).then_inc(xin_sems[0], 16)
            sync.dma_start(embT2_s[:, 0:512], embT2[:, 0:512]) \
                .then_inc(cemb_sems[0], 16)
            sync.dma_start(embT2_s[:, 512:1024], embT2[:, 512:1024]) \
                .then_inc(cemb_sems[1], 16)
            sync.dma_start(esq_s[:], esq3[:]).then_inc(cbias_sem, 16)
            sync.dma_start(ones_s[:], ones3[:]).then_inc(cbias_sem, 16)
            for i in range(1, NT + SLAG):
                if i < NT:
                    if i >= NX:
                        # slot reuse: PE must be done with tile i-NX
                        sync.wait_ge(pe_sem, 2 * (i - NX + 1))
                    sync.dma_start(x_sb[i % NX][:], xT[:, i * 128:(i + 1) * 128]) \
                        .then_inc(xin_sems[i % NX], 16)
                j = i - SLAG
                if j == NT - 1:
                    # last tile's indices are final before its gather completes:
                    # store the tail of ind ahead of the last q store
                    sync.wait_ge(dve_sem, NT)
                    sync.dma_start(ind_out[:, NT // 2:, :],
                                   idx_stage[:, NT // 2 * 8:].bitcast(I32)) \
                        .then_inc(out_sem, 16)
                if 0 <= j < NT:
                    sync.wait_ge(gat_sems[j % NQ], 16 * (j // NQ + 1))
                    sync.dma_start(q_out[j * 128:(j + 1) * 128, :],
                                   q_sb[j % NQ][:]).then_inc(qout_sems[j % NQ], 16)
                if j == NT // 2:
                    # first half of the indices is final; overlap its store
                    sync.wait_ge(dve_sem, NT // 2)
                    sync.dma_start(ind_out[:, :NT // 2, :],
                                   idx_stage[:, :NT // 2 * 8].bitcast(I32)) \
                        .then_inc(out_sem, 16)

        @block.tensor
        def _(tensor):
            tensor.wait_ge(warm_sem, 1)
            for _ in range(8):
                nc.tensor.matmul(d_ps[0][0:1, 0:512], lhsT=pe_warm[:, 0:1],
                                 rhs=pe_warm[:], start=True, stop=True)
            tensor.wait_ge(cbias_sem, 32)
            for i in range(NT):
                tensor.wait_ge(xin_sems[i % NX], 16 * (i // NX + 1))
                if i >= NPS:
                    # psum slot reuse: both extraction halves of tile i-NPS done
                    tensor.wait_ge(ext_sem, i - NPS + 1)
                ps = d_ps[i % NPS]
                xs = x_sb[i % NX]
                for j in range(2):
                    if i == 0:
                        tensor.wait_ge(cemb_sems[j], 16)
                    sl = slice(j * 512, (j + 1) * 512)
                    nc.tensor.matmul(ps[:, sl], lhsT=xs[:], rhs=embT2_s[:, sl],
                                     start=True, stop=False)
                    nc.tensor.matmul(ps[:, sl], lhsT=ones_s[:],
                                     rhs=esq_s[:, sl], start=False, stop=True) \
                        .then_inc(pe_sem, 1)

        @block.scalar
        def _(scalar):
            scalar.wait_ge(cbias_sem, 32)
            nc.scalar.copy(act_warm[:], ones_s[0:1, 0:1])
            for i in range(NT):
                scalar.wait_ge(pe_sem, 2 * i + 1)
                if i >= ND:
                    # d slot reuse: DVE must be done with tile i-ND
                    scalar.wait_ge(dve_sem, i - ND + 1)
                nc.scalar.copy(d_sb[i % ND][:, 0:512], d_ps[i % NPS][:, 0:512])
                scalar.wait_ge(pe_sem, 2 * i + 2)
                nc.scalar.copy(d_sb[i % ND][:, 512:1024],
                               d_ps[i % NPS][:, 512:1024]).then_inc(ext_sem, 1)

        @block.vector
        def _(vector):
            # software-pipelined: max(i) runs between max_index(i-1)'s producer
            # max(i-1) and its read of mx_sb -- the intervening ~1us op drains
            # the DVE pipeline, so no explicit drain is needed.
            for i in range(NT):
                vector.wait_ge(ext_sem, i + 1)
                if i >= 4:
                    # mx slot WAR: max_index(i-4) must have read mx_sb[i%4]
                    vector.wait_ge(dve_sem, i - 3)
                nc.vector.max(out=mx_sb[i % 4][:], in_=d_sb[i % ND][:]) \
                    .then_inc(mx_sem, 1)
                if i >= 1:
                    j = i - 1
                    vector.wait_ge(mx_sem, j + 1)
                    nc.vector.max_index(out=idx_stage[:, 8 * j:8 * (j + 1)],
                                        in_max=mx_sb[j % 4][:],
                                        in_values=d_sb[j % ND][:]) \
                        .then_inc(dve_sem, 1)
            j = NT - 1
            vector.wait_ge(mx_sem, j + 1)
            nc.vector.max_index(out=idx_stage[:, 8 * j:8 * (j + 1)],
                                in_max=mx_sb[j % 4][:],
                                in_values=d_sb[j % ND][:]) \
                .then_inc(dve_sem, 1)

        @block.gpsimd
        def _(gpsimd):
            nc.gpsimd.memset(pe_warm[:], 0.0).then_inc(warm_sem, 1)
            for i in range(NT):
                gpsimd.wait_ge(dve_sem, i + 1)
                if i >= NQ:
                    gpsimd.wait_ge(qout_sems[i % NQ], 16 * (i // NQ))
                nc.gpsimd.indirect_dma_start(
                    out=q_sb[i % NQ][:], out_offset=None, in_=embed[:],
                    in_offset=bass.IndirectOffsetOnAxis(
                        ap=idx_stage[:, 8 * i:8 * i + 1], axis=0)
                ).then_inc(gat_sems[i % NQ], 16)
    return nc


_CACHE = {}


def _get_nc():
    if "nc" not in _CACHE:
        nc = bass.Bass()
        build(nc)
        _CACHE["nc"] = nc
    return _CACHE["nc"]


def kernel(x, embed):
    x = np.asarray(x, dtype=np.float32)
    embed = np.asarray(embed, dtype=np.float32)
    assert x.shape == (B, TOK, C) and embed.shape == (K, C)

    embT2 = np.ascontiguousarray((2.0 * embed).T).astype(np.float32)
    esq = (embed.astype(np.float64) ** 2).sum(1)
    # exact -esq as sum of three bf16 terms
    t = -esq
    e1 = t.astype(ml_dtypes.bfloat16)
    t1 = t - e1.astype(np.float64)
    e2 = t1.astype(ml_dtypes.bfloat16)
    t2 = t1 - e2.astype(np.float64)
    e3 = t2.astype(ml_dtypes.bfloat16)
    esq3 = np.stack([e1, e2, e3]).astype(ml_dtypes.bfloat16)
    ones3 = np.ones((3, 128), ml_dtypes.bfloat16)

    nc = _get_nc()
    in_maps = []
    for b in range(B):
        xTb = np.ascontiguousarray(x[b].T)
        in_maps.append(dict(xT=xTb, embT2=embT2, esq3=esq3, ones3=ones3,
                            embed=embed))
    res = run_bass_kernel_spmd(nc, in_maps, core_ids=list(range(B)))

    quant = np.empty((B, TOK, C), np.float32)
    ind = np.empty((B, TOK), np.int32)
    for b in range(B):
        out = res.results[b]
        quant[b] = out["q"]
        ind[b] = out["ind"][:, :, 0].T.reshape(TOK)
    return quant, ind
